# revision 16
# baseline (speedup 1.0000x reference)
"""Trainium2 Bass kernel for nn_Attention_57037165691498.

8 NeuronCores, SPMD, zero collectives:
  - Both time-recurrences (query-attention GRU, decoder GRU) are sequential
    over T; per-step cross-core collectives would dominate, so every core
    runs the scans redundantly.
  - The big decoder MLP (~30 G-MAC over T*N rows) is sharded over the N
    (objects) dimension via a host-sliced per-core input; host concatenates
    pred slices.
  - Critical-path shrinkers: x_fix GEMM contributions batched (P1/P2);
    W_ih_query @ query folded to (W_ih_query @ Wq.T) @ hid; decoder GRU
    input GEMM batched over T; single ACT table (Sigmoid) for every
    transcendental: tanh(x)=2*sig(2x)-1, softmax-exp via s/(1-s).
"""

import numpy as np

import concourse.bass as bass
import concourse.mybir as mybir
import concourse.tile as tile
from concourse import bacc, bass_utils
from concourse.bass import ds

F32 = mybir.dt.float32
P = 128
AF = mybir.ActivationFunctionType
OP = mybir.AluOpType
AX = mybir.AxisListType

N_CORES = 8
BODY = 16


def _chunks(w):
    K, M = w.shape
    assert K % P == 0
    return np.ascontiguousarray(w.reshape(K // P, P, M).transpose(1, 0, 2))


def build_program(T, HZ, NLOC, add_c0):
    N, A, NH = 64, 64, 8
    QG, DG = 256, 256
    assert HZ <= BODY and T % BODY == 0

    nc = bacc.Bacc("TRN2", target_bir_lowering=False, debug=False)

    def din(name, shape):
        return nc.dram_tensor(name, shape, F32, kind="ExternalInput").ap()

    def dout(name, shape):
        return nc.dram_tensor(name, shape, F32, kind="ExternalOutput").ap()

    def dtmp(name, shape):
        return nc.dram_tensor(name, shape, F32, kind="Internal").ap()

    x_att_d = din("x_att", [T, N, A])
    x_fix_d = din("x_fix", [T, 64])
    x_att_nloc_d = din("x_att_nloc", [T, NLOC, A])
    ident_d = din("ident", [P, P])
    sel_d = din("sel", [64, P])  # sel[a, p] = (p == a + 64)
    c0_d = din("c0", [P, 6])
    wg_d = din("wg", [P, 8, 768])
    whid_d = din("whid", [P, 6, 256])
    wq_d = din("wq", [P, 2, 512])
    wfixih_d = din("wfixih", [64, 768])
    wfixhid_d = din("wfixhid", [64, 256])
    p1b_d = din("p1b", [P, 6])
    p2b_d = din("p2b", [P, 2])
    wdfix_d = din("wdfix", [64, 768])
    wdav_d = din("wdav", [P, 4, 768])
    wdhh_d = din("wdhh", [P, 2, 768])
    wfh_d = din("wfh", [P, 2, 256])
    wffix_d = din("wffix", [64, 256])
    wfav_d = din("wfav", [P, 4, 256])
    wa_d = din("wa", [64, 256])
    w2t_d = din("w2t", [P, 2, 64])
    dhb_d = din("dhb", [P, 2])
    b2b_d = din("b2b", [P, 64])

    import os
    DBG = bool(int(os.environ.get("KDBG", "0")))
    if DBG:
        dbg_h = dout("dbg_h", [P, 2])
        dbg_hid = dout("dbg_hid", [P, 2])
        dbg_qT = dout("dbg_qT", [64, 8])
        dbg_av = dout("dbg_av", [P, 4])
        dbg_gi = dout("dbg_gi", [P, 12])
        dbg_S = dout("dbg_S", [8, 64])
        dbg_aT = dout("dbg_aT", [64, 64])
    pred_d = dout("pred", [T, NLOC, A])
    attw_d = dout("attw", [T, NH, N])
    qgru_d = dout("q_gru_n", [QG])
    dgru_d = dout("dec_gru_n", [DG])
    qn_d = dout("query_n", [NH * A])

    p1_d = dtmp("p1s", [P, 6, T])
    p2_d = dtmp("p2s", [P, 2, T])
    attwT_d = dtmp("attwTs", [64, NH, T])
    decatT_d = dtmp("decatTs", [P, 4, T])
    gid_d = dtmp("gids", [P, 6, T])
    hdT_d = dtmp("hdTs", [P, 2, T])

    xa_rows = x_att_d.rearrange("t n a -> (t n) a")

    with tile.TileContext(nc) as tc:
        with tc.tile_pool(name="st", bufs=1) as st:
            ident = st.tile([P, P], F32, tag="ident")
            sel = st.tile([64, P], F32, tag="sel")
            nc.sync.dma_start(ident[:], ident_d[:])
            nc.sync.dma_start(sel[:], sel_d[:])
            xfixT = st.tile([64, T], F32, tag="xfixT")
            h_sb = st.tile([P, 2], F32, tag="h_sb")
            hid_sb = st.tile([P, 2], F32, tag="hid_sb")
            qT_sb = st.tile([64, NH], F32, tag="qT_sb")
            hd_state = st.tile([P, 2], F32, tag="hd_state")
            nc.vector.memset(h_sb[:], 0.0)
            nc.vector.memset(hid_sb[:], 0.0)
            nc.vector.memset(qT_sb[:], 1.0)
            nc.vector.memset(hd_state[:], 0.0)

            # ============ prologue: x_fixT, P1, P2 ============
            with tc.tile_pool(name="pro", bufs=2) as pro, \
                 tc.tile_pool(name="prop", bufs=2, space="PSUM") as prop:
                wfi = pro.tile([64, 768], F32, tag="wfi")
                wfh2 = pro.tile([64, 256], F32, tag="wfh2")
                p1b = pro.tile([P, 6], F32, tag="p1b")
                p2b = pro.tile([P, 2], F32, tag="p2b")
                c0 = pro.tile([P, 6], F32, tag="c0")
                nc.sync.dma_start(wfi[:], wfixih_d[:])
                nc.sync.dma_start(wfh2[:], wfixhid_d[:])
                nc.sync.dma_start(p1b[:], p1b_d[:])
                nc.sync.dma_start(p2b[:], p2b_d[:])
                nc.sync.dma_start(c0[:], c0_d[:])
                for s in range((T + P - 1) // P):
                    rows = min(P, T - s * P)
                    xf = pro.tile([P, 64], F32, tag="xf")
                    nc.sync.dma_start(xf[:rows], x_fix_d[s * P:s * P + rows])
                    pt = prop.tile([64, P], F32, tag="pt")
                    nc.tensor.transpose(pt[:, :rows], xf[:rows], ident[:rows, :rows])
                    nc.vector.tensor_copy(xfixT[:, s * P:s * P + rows], pt[:, :rows])
                NBK = min(512, T)
                for nb in range(T // NBK):
                    sl = slice(nb * NBK, (nb + 1) * NBK)
                    for m in range(6):
                        pg = prop.tile([P, NBK], F32, tag="pg")
                        nc.tensor.matmul(pg[:], wfi[:, m * P:(m + 1) * P],
                                         xfixT[:, sl], start=True, stop=True)
                        sb = pro.tile([P, NBK], F32, tag="sb1")
                        nc.vector.tensor_scalar_add(sb[:], pg[:], p1b[:, m:m + 1])
                        if add_c0 and nb == 0:
                            nc.vector.tensor_add(sb[:, 0:1], sb[:, 0:1],
                                                 c0[:, m:m + 1])
                        nc.sync.dma_start(p1_d[:, m, sl], sb[:])
                    for m in range(2):
                        pg = prop.tile([P, NBK], F32, tag="pg")
                        nc.tensor.matmul(pg[:], wfh2[:, m * P:(m + 1) * P],
                                         xfixT[:, sl], start=True, stop=True)
                        sb = pro.tile([P, NBK], F32, tag="sb1")
                        nc.vector.tensor_scalar_add(sb[:], pg[:], p2b[:, m:m + 1])
                        nc.sync.dma_start(p2_d[:, m, sl], sb[:])

            # ============ phase 1: query scan ============
            with tc.tile_pool(name="qw", bufs=1) as qw, \
                 tc.tile_pool(name="qs", bufs=2) as qs, \
                 tc.tile_pool(name="qr", bufs=2) as qr, \
                 tc.tile_pool(name="psA", bufs=2, space="PSUM") as psA, \
                 tc.tile_pool(name="psB", bufs=1, space="PSUM") as psB, \
                 tc.tile_pool(name="psD", bufs=1, space="PSUM") as psD, \
                 tc.tile_pool(name="psE", bufs=1, space="PSUM") as psE:
                wg = qw.tile([P, 8, 768], F32, tag="wg")
                whid = qw.tile([P, 6, 256], F32, tag="whid")
                wq = qw.tile([P, 2, 512], F32, tag="wq")
                nc.sync.dma_start(wg[:], wg_d[:])
                nc.sync.dma_start(whid[:], whid_d[:])
                nc.sync.dma_start(wq[:], wq_d[:])

                def q_step(p1t, p2t, att_nat, attT, attw_slot, attwT_slot,
                           dbg=False, dbg2=False):
                    sS = psB.tile([8, 64], F32, tag="sS")
                    nc.tensor.matmul(sS[:], qT_sb[:], attT,
                                     start=True, stop=True)
                    if dbg2:
                        dtmp_ = qs.tile([8, 64], F32, tag="dbgS")
                        nc.vector.tensor_copy(dtmp_[:], sS[:])
                        nc.sync.dma_start(dbg_S[:], dtmp_[:])
                        nc.sync.dma_start(dbg_aT[:], attT)
                    hq = psA.tile([P, 10], F32, tag="hq")
                    aA = psA.tile([P, 18], F32, tag="aA")
                    for m in range(6):
                        for k in (4, 5):
                            nc.tensor.matmul(aA[:, 12 + m:13 + m],
                                             wg[:, k, m * P:(m + 1) * P],
                                             hid_sb[:, k - 4:k - 3],
                                             start=(k == 4), stop=(k == 5))
                        for k in (6, 7):
                            nc.tensor.matmul(aA[:, 6 + m:7 + m],
                                             wg[:, k, m * P:(m + 1) * P],
                                             h_sb[:, k - 6:k - 5],
                                             start=(k == 6), stop=(k == 7))
                    # softmax over n (free dim), heads on partitions 0..7
                    mxs = qs.tile([8, 1], F32, tag="mxs")
                    nc.vector.reduce_max(mxs[:], sS[:], axis=AX.X)
                    nc.vector.tensor_scalar_mul(mxs[:], mxs[:], -0.125)
                    sg = qs.tile([8, 64], F32, tag="sg")
                    nc.scalar.activation(sg[:], sS[:], AF.Sigmoid,
                                         bias=mxs[:], scale=0.125)
                    om = qs.tile([8, 64], F32, tag="om")
                    nc.vector.tensor_scalar(om[:], sg[:], -1.0, 1.0, OP.mult, OP.add)
                    nc.vector.reciprocal(om[:], om[:])
                    wv = qs.tile([8, 64], F32, tag="wv")
                    nc.vector.tensor_mul(wv[:], sg[:], om[:])
                    sm = qs.tile([8, 1], F32, tag="sm")
                    nc.vector.reduce_sum(sm[:], wv[:], axis=AX.X)
                    nc.vector.reciprocal(sm[:], sm[:])
                    nc.vector.tensor_scalar_mul(attw_slot, wv[:], sm[:])
                    # attwT via PE transpose -> SBUF slot
                    dD = psD.tile([P, 24], F32, tag="dD")
                    nc.tensor.transpose(dD[0:64, 0:8], attw_slot, ident[0:8, 0:8])
                    nc.vector.tensor_copy(attwT_slot, dD[0:64, 0:8])
                    # avT = att^T @ attwT  [64a, 8h]
                    nc.tensor.matmul(dD[0:64, 8:16], att_nat, attwT_slot,
                                     start=True, stop=True)
                    avT = qs.tile([64, 8], F32, tag="avT")
                    nc.vector.tensor_copy(avT[:], dD[0:64, 8:16])
                    # pack avT -> av [128, 4] via selector matmuls
                    nc.tensor.matmul(dD[:, 16:20], ident[0:64, :], avT[:, 0:8:2],
                                     start=True, stop=False)
                    nc.tensor.matmul(dD[:, 16:20], sel[:], avT[:, 1:8:2],
                                     start=False, stop=True)
                    av = qs.tile([P, 4], F32, tag="av")
                    nc.vector.tensor_copy(av[:], dD[:, 16:20])
                    # gi av part
                    for m in range(6):
                        for k in range(4):
                            nc.tensor.matmul(aA[:, m:m + 1],
                                             wg[:, k, m * P:(m + 1) * P],
                                             av[:, k:k + 1],
                                             start=(k == 0), stop=(k == 3))
                    # hid av part (overlaps gates)
                    for m in range(2):
                        for k in range(2, 6):
                            nc.tensor.matmul(hq[:, m:m + 1],
                                             whid[:, k, m * P:(m + 1) * P],
                                             av[:, k - 2:k - 1],
                                             start=(k == 2), stop=(k == 5))
                    # gates
                    prerz = qs.tile([P, 4], F32, tag="prerz")
                    nc.vector.tensor_add(prerz[:], aA[:, 0:4], p1t[:, 0:4])
                    nc.vector.tensor_add(prerz[:], prerz[:], aA[:, 6:10])
                    nc.vector.tensor_add(prerz[:], prerz[:], aA[:, 12:16])
                    rz = qs.tile([P, 4], F32, tag="rz")
                    nc.scalar.activation(rz[:], prerz[:], AF.Sigmoid)
                    pren = qs.tile([P, 2], F32, tag="pren")
                    nc.vector.tensor_mul(pren[:], rz[:, 0:2], aA[:, 10:12])
                    nc.vector.tensor_add(pren[:], pren[:], aA[:, 4:6])
                    nc.vector.tensor_add(pren[:], pren[:], aA[:, 16:18])
                    nc.vector.tensor_add(pren[:], pren[:], p1t[:, 4:6])
                    sn = qs.tile([P, 2], F32, tag="sn")
                    nc.scalar.activation(sn[:], pren[:], AF.Sigmoid, scale=2.0)
                    u = qs.tile([P, 2], F32, tag="u")
                    nc.vector.tensor_scalar(u[:], sn[:], -2.0, 1.0, OP.mult, OP.add)
                    nc.vector.tensor_add(u[:], u[:], h_sb[:])
                    nc.vector.tensor_mul(u[:], rz[:, 2:4], u[:])
                    tn = qs.tile([P, 2], F32, tag="tn")
                    nc.vector.tensor_scalar(tn[:], sn[:], 2.0, -1.0, OP.mult, OP.add)
                    nc.vector.tensor_add(h_sb[:], u[:], tn[:])
                    # hid h2 part + relu
                    for m in range(2):
                        for k in range(2):
                            nc.tensor.matmul(hq[:, 6 + m:7 + m],
                                             whid[:, k, m * P:(m + 1) * P],
                                             h_sb[:, k:k + 1],
                                             start=(k == 0), stop=(k == 1))
                    nc.vector.tensor_add(hid_sb[:], hq[:, 0:2], p2t[:])
                    nc.vector.tensor_add(hid_sb[:], hid_sb[:], hq[:, 6:8])
                    nc.vector.tensor_scalar_max(hid_sb[:], hid_sb[:], 0.0)
                    # query chunks then pack to qT [64, 8]
                    for m in range(4):
                        for k in range(2):
                            nc.tensor.matmul(hq[:, 2 + m:3 + m],
                                             wq[:, k, m * P:(m + 1) * P],
                                             hid_sb[:, k:k + 1],
                                             start=(k == 0), stop=(k == 1))
                    q_sb = qs.tile([P, 4], F32, tag="q_sb")
                    nc.vector.tensor_copy(q_sb[:], hq[:, 2:6])
                    qTp = psD.tile([64, 8], F32, tag="qTp")
                    nc.tensor.matmul(qTp[:, 0:8:2], ident[:, 0:64], q_sb[:],
                                     start=True, stop=True)
                    nc.tensor.matmul(qTp[:, 1:8:2], ident[:, 64:128], q_sb[:],
                                     start=True, stop=True)
                    nc.vector.tensor_copy(qT_sb[:], qTp[:])
                    if dbg:
                        ds_ = qs.tile([P, 12], F32, tag="dbgavc")
                        nc.vector.tensor_copy(ds_[:, 0:12], aA[:, 0:12])
                        nc.sync.dma_start(dbg_gi[:], ds_[:, 0:12])
                        nc.sync.dma_start(dbg_av[:], av[:])
                        nc.sync.dma_start(dbg_h[:], h_sb[:])
                        nc.sync.dma_start(dbg_hid[:], hid_sb[:])
                        nc.sync.dma_start(dbg_qT[:], qT_sb[:])

                with tc.For_i(0, T, BODY) as i:
                    p1r = qr.tile([P, 6, BODY], F32, tag="p1r")
                    p2r = qr.tile([P, 2, BODY], F32, tag="p2r")
                    nc.sync.dma_start(p1r[:], p1_d[:, :, ds(i, BODY)])
                    nc.sync.dma_start(p2r[:], p2_d[:, :, ds(i, BODY)])
                    attwr = qr.tile([8, BODY, 64], F32, tag="attwr")
                    attwTr = qr.tile([64, NH, BODY], F32, tag="attwTr")
                    attTs = []
                    for jp in range(BODY // 2):
                        sl_ = qr.tile([P, 64], F32, tag=f"slab{jp % 2}")
                        nc.sync.dma_start(sl_[:], xa_rows[ds(i * 64 + jp * 128, P)])
                        pE = psE.tile([64, P], F32, tag="pE")
                        nc.tensor.transpose(pE[:], sl_[:], ident[:])
                        aT = qr.tile([64, P], F32, tag=f"attT{jp % 4}")
                        nc.vector.tensor_copy(aT[:], pE[:])
                        attTs.append(aT)
                    atts = []
                    for j in range(BODY):
                        an = qr.tile([64, 64], F32, tag=f"attn{j % 4}")
                        nc.sync.dma_start(an[:], xa_rows[ds(i * 64 + j * 64, 64)])
                        atts.append(an)
                    for j in range(BODY):
                        aT = attTs[j // 2]
                        q_step(p1r[:, :, j], p2r[:, :, j], atts[j][:],
                               aT[:, (j % 2) * 64:(j % 2) * 64 + 64],
                               attwr[:, j, :], attwTr[:, :, j],
                               dbg=(DBG and j == 0), dbg2=(DBG and j == 1))
                    nc.sync.dma_start(
                        attw_d[ds(i, BODY)].rearrange("t h n -> h t n"), attwr[:])
                    nc.sync.dma_start(attwT_d[:, :, ds(i, BODY)], attwTr[:])

                nc.sync.dma_start(qgru_d.rearrange("(c p) -> p c", p=P), h_sb[:])
                nc.sync.dma_start(qn_d.rearrange("(h a) -> a h", a=64), qT_sb[:])

            # ============ phase D1: dec_att_aT ============
            with tc.tile_pool(name="d1", bufs=2) as d1, \
                 tc.tile_pool(name="d1p", bufs=2, space="PSUM") as d1p:

                def d1_step(att_nat, awT, ring, j):
                    pD = d1p.tile([P, 12], F32, tag="pD")
                    nc.tensor.matmul(pD[0:64, 0:8], att_nat, awT,
                                     start=True, stop=True)
                    pd_sb = d1.tile([64, 8], F32, tag="pd_sb")
                    nc.vector.tensor_copy(pd_sb[:], pD[0:64, 0:8])
                    nc.tensor.matmul(pD[:, 8:12], ident[0:64, :], pd_sb[:, 0:8:2],
                                     start=True, stop=False)
                    nc.tensor.matmul(pD[:, 8:12], sel[:], pd_sb[:, 1:8:2],
                                     start=False, stop=True)
                    nc.vector.tensor_copy(ring[:, :, j], pD[:, 8:12])

                def d1_body(i, dyn):
                    awr = d1.tile([64, NH, BODY], F32, tag="awr")
                    if dyn:
                        nc.sync.dma_start(awr[:], attwT_d[:, :, ds(i - HZ, BODY)])
                    else:
                        if HZ < BODY:
                            nc.sync.dma_start(awr[:, :, HZ:BODY],
                                              attwT_d[:, :, 0:BODY - HZ])
                        for j in range(HZ):
                            nc.sync.dma_start(awr[:, :, j:j + 1],
                                              attwT_d[:, :, 0:1])
                    ring = d1.tile([P, 4, BODY], F32, tag="datr")
                    for j in range(BODY):
                        at = d1.tile([64, 64], F32, tag=f"dat{j % 4}")
                        if dyn:
                            nc.sync.dma_start(at[:], xa_rows[ds(i * 64 + j * 64, 64)])
                        else:
                            nc.sync.dma_start(at[:], x_att_d[j])
                        d1_step(at[:], awr[:, :, j], ring, j)
                    if dyn:
                        nc.sync.dma_start(decatT_d[:, :, ds(i, BODY)], ring[:])
                    else:
                        nc.sync.dma_start(decatT_d[:, :, 0:BODY], ring[:])

                d1_body(0, dyn=False)
                if T > BODY:
                    with tc.For_i(BODY, T, BODY) as i:
                        d1_body(i, dyn=True)

            # ============ phase D2: gi_dec GEMM ============
            with tc.tile_pool(name="d2", bufs=2) as d2, \
                 tc.tile_pool(name="d2p", bufs=2, space="PSUM") as d2p:
                wdfix = d2.tile([64, 768], F32, tag="wdfix")
                wdav = d2.tile([P, 4, 768], F32, tag="wdav")
                nc.sync.dma_start(wdfix[:], wdfix_d[:])
                nc.sync.dma_start(wdav[:], wdav_d[:])
                NBK = min(512, T)
                for nb in range(T // NBK):
                    sl = slice(nb * NBK, (nb + 1) * NBK)
                    dat = d2.tile([P, 4, NBK], F32, tag="dat2")
                    nc.sync.dma_start(dat[:], decatT_d[:, :, sl])
                    for m in range(6):
                        pg = d2p.tile([P, NBK], F32, tag="pg2")
                        nc.tensor.matmul(pg[:], wdfix[:, m * P:(m + 1) * P],
                                         xfixT[:, sl], start=True, stop=False)
                        for k in range(4):
                            nc.tensor.matmul(pg[:], wdav[:, k, m * P:(m + 1) * P],
                                             dat[:, k, :], start=False,
                                             stop=(k == 3))
                        sb = d2.tile([P, NBK], F32, tag="sb2")
                        nc.vector.tensor_copy(sb[:], pg[:])
                        nc.sync.dma_start(gid_d[:, m, sl], sb[:])

            # ============ phase D3: decoder scan ============
            with tc.tile_pool(name="d3w", bufs=1) as d3w, \
                 tc.tile_pool(name="d3", bufs=2) as d3, \
                 tc.tile_pool(name="d3p", bufs=2, space="PSUM") as d3p:
                wdhh = d3w.tile([P, 2, 768], F32, tag="wdhh")
                nc.sync.dma_start(wdhh[:], wdhh_d[:])
                with tc.For_i(0, T, BODY) as i:
                    gir = d3.tile([P, 6, BODY], F32, tag="gir")
                    nc.sync.dma_start(gir[:], gid_d[:, :, ds(i, BODY)])
                    hdr = d3.tile([P, 2, BODY], F32, tag="hdr")
                    for j in range(BODY):
                        hprev = hd_state[:] if j == 0 else hdr[:, :, j - 1]
                        aG = d3p.tile([P, 6], F32, tag="aG")
                        for m in range(6):
                            for k in range(2):
                                nc.tensor.matmul(aG[:, m:m + 1],
                                                 wdhh[:, k, m * P:(m + 1) * P],
                                                 hprev[:, k:k + 1],
                                                 start=(k == 0), stop=(k == 1))
                        prerz = d3.tile([P, 4], F32, tag="dprerz")
                        nc.vector.tensor_add(prerz[:], gir[:, 0:4, j], aG[:, 0:4])
                        rz = d3.tile([P, 4], F32, tag="drz")
                        nc.scalar.activation(rz[:], prerz[:], AF.Sigmoid)
                        pren = d3.tile([P, 2], F32, tag="dpren")
                        nc.vector.tensor_mul(pren[:], rz[:, 0:2], aG[:, 4:6])
                        nc.vector.tensor_add(pren[:], pren[:], gir[:, 4:6, j])
                        sn = d3.tile([P, 2], F32, tag="dsn")
                        nc.scalar.activation(sn[:], pren[:], AF.Sigmoid, scale=2.0)
                        u = d3.tile([P, 2], F32, tag="du")
                        nc.vector.tensor_scalar(u[:], sn[:], -2.0, 1.0,
                                                OP.mult, OP.add)
                        nc.vector.tensor_add(u[:], u[:], hprev)
                        nc.vector.tensor_mul(u[:], rz[:, 2:4], u[:])
                        tn = d3.tile([P, 2], F32, tag="dtn")
                        nc.vector.tensor_scalar(tn[:], sn[:], 2.0, -1.0,
                                                OP.mult, OP.add)
                        nc.vector.tensor_add(hdr[:, :, j], u[:], tn[:])
                    nc.vector.tensor_copy(hd_state[:], hdr[:, :, BODY - 1])
                    nc.sync.dma_start(hdT_d[:, :, ds(i, BODY)], hdr[:])
                nc.sync.dma_start(dgru_d.rearrange("(c p) -> p c", p=P),
                                  hd_state[:])

            # ============ phase M: decoder MLP (n-sharded) ============
            with tc.tile_pool(name="m", bufs=3) as mp_, \
                 tc.tile_pool(name="mw", bufs=1) as mw, \
                 tc.tile_pool(name="mpp", bufs=2, space="PSUM") as mpp:
                wfh = mw.tile([P, 2, 256], F32, tag="wfh")
                wffix = mw.tile([64, 256], F32, tag="wffix")
                wfav = mw.tile([P, 4, 256], F32, tag="wfav")
                wa = mw.tile([64, 256], F32, tag="wa")
                w2t = mw.tile([P, 2, 64], F32, tag="w2t")
                dhb = mw.tile([P, 2], F32, tag="dhb")
                b2b = mw.tile([P, 64], F32, tag="b2b")
                for tl, dr in [(wfh, wfh_d), (wffix, wffix_d), (wfav, wfav_d),
                               (wa, wa_d), (w2t, w2t_d), (dhb, dhb_d),
                               (b2b, b2b_d)]:
                    nc.sync.dma_start(tl[:], dr[:])
                hdT = mw.tile([P, 2, T], F32, tag="hdT")
                datT = mw.tile([P, 4, T], F32, tag="datT")
                nc.sync.dma_start(hdT[:], hdT_d[:])
                nc.sync.dma_start(datT[:], decatT_d[:])
                g1 = mw.tile([P, 2, T], F32, tag="g1")
                NBK = min(512, T)
                for nb in range(T // NBK):
                    sl = slice(nb * NBK, (nb + 1) * NBK)
                    for m in range(2):
                        pg = mpp.tile([P, NBK], F32, tag="pgm")
                        for k in range(2):
                            nc.tensor.matmul(pg[:], wfh[:, k, m * P:(m + 1) * P],
                                             hdT[:, k, sl], start=(k == 0),
                                             stop=False)
                        nc.tensor.matmul(pg[:], wffix[:, m * P:(m + 1) * P],
                                         xfixT[:, sl], start=False, stop=False)
                        for k in range(4):
                            nc.tensor.matmul(pg[:], wfav[:, k, m * P:(m + 1) * P],
                                             datT[:, k, sl], start=False,
                                             stop=(k == 3))
                        nc.vector.tensor_scalar_add(g1[:, m, sl], pg[:],
                                                    dhb[:, m:m + 1])
                RT = T * NLOC
                xaT = mw.tile([64, RT], F32, tag="xaT")
                xan_rows = x_att_nloc_d.rearrange("t n a -> (t n) a")
                for s in range(RT // P):
                    xf = mp_.tile([P, 64], F32, tag="xfm")
                    nc.sync.dma_start(xf[:], xan_rows[s * P:(s + 1) * P])
                    pt = mpp.tile([64, P], F32, tag="ptm")
                    nc.tensor.transpose(pt[:], xf[:], ident[:])
                    nc.vector.tensor_copy(xaT[:, s * P:(s + 1) * P], pt[:])
                TPB = P // NLOC
                pred_rows = pred_d.rearrange("t n a -> (t n) a")
                for blk in range(RT // P):
                    t0 = blk * TPB
                    g2 = mpp.tile([P, 2, P], F32, tag="g2")
                    for m in range(2):
                        nc.tensor.matmul(g2[:, m, :], wa[:, m * P:(m + 1) * P],
                                         xaT[:, blk * P:(blk + 1) * P],
                                         start=True, stop=True)
                    hidT = mp_.tile([P, 2, P], F32, tag="hidT")
                    for m in range(2):
                        nc.vector.tensor_add(
                            hidT[:, m, :].rearrange("p (t n) -> p t n", n=NLOC),
                            g2[:, m, :].rearrange("p (t n) -> p t n", n=NLOC),
                            g1[:, m, t0:t0 + TPB, None].to_broadcast(
                                (P, TPB, NLOC)))
                        nc.vector.tensor_scalar_max(hidT[:, m, :],
                                                    hidT[:, m, :], 0.0)
                    pp = mpp.tile([P, 64], F32, tag="pp")
                    for m in range(2):
                        nc.tensor.matmul(pp[:], hidT[:, m, :], w2t[:, m, :],
                                         start=(m == 0), stop=(m == 1))
                    po = mp_.tile([P, 64], F32, tag="po")
                    nc.vector.tensor_add(po[:], pp[:], b2b[:])
                    nc.vector.tensor_scalar_max(po[:], po[:], 0.0)
                    nc.sync.dma_start(pred_rows[blk * P:(blk + 1) * P], po[:])
    return nc


def _prep(inputs, NLOC, n0):
    gw = {k: (np.asarray(v, dtype=np.float32) if hasattr(v, "shape") or
              isinstance(v, (list, tuple)) else v) for k, v in inputs.items()}
    w_ih = np.asarray(gw["q_gru_w_ih"], np.float32)
    w_hh = np.asarray(gw["q_gru_w_hh"], np.float32)
    wq = np.asarray(gw["weight_q"], np.float32)
    whid = np.asarray(gw["q_hid_w"], np.float32)
    wfold = w_ih[:, 576:1088] @ wq.T
    wg = np.concatenate([
        _chunks(np.ascontiguousarray(w_ih[:, 64:576].T)),
        _chunks(np.ascontiguousarray(wfold.T)),
        _chunks(np.ascontiguousarray(w_hh.T)),
    ], axis=1)
    whid_l = np.concatenate([
        _chunks(np.ascontiguousarray(whid[:, 0:256].T)),
        _chunks(np.ascontiguousarray(whid[:, 320:832].T)),
    ], axis=1)
    wq_l = _chunks(wq)
    c0 = np.ascontiguousarray(
        (w_ih[:, 576:1088] @ np.ones((512,), np.float32)).reshape(6, P).T)
    b_ih = np.asarray(gw["q_gru_b_ih"], np.float32)
    b_hh = np.asarray(gw["q_gru_b_hh"], np.float32)
    assert np.all(b_hh[512:] == 0), "nonzero n-gate b_hh unsupported"
    p1b = np.ascontiguousarray(
        (b_ih + np.concatenate([b_hh[:512], np.zeros(256, np.float32)])
         ).reshape(6, P).T)
    p2b = np.ascontiguousarray(
        np.asarray(gw["q_hid_b"], np.float32).reshape(2, P).T)
    dw_ih = np.asarray(gw["dec_gru_w_ih"], np.float32)
    dw_hh = np.asarray(gw["dec_gru_w_hh"], np.float32)
    assert np.all(np.asarray(gw["dec_gru_b_ih"]) == 0)
    assert np.all(np.asarray(gw["dec_gru_b_hh"]) == 0)
    dh_w = np.asarray(gw["dec_hid_w"], np.float32)
    sel = np.zeros((64, P), np.float32)
    for a in range(64):
        sel[a, a + 64] = 1.0
    x_att = np.asarray(gw["x_att"], np.float32)
    return {
        "x_att": x_att,
        "x_fix": np.asarray(gw["x_fix"], np.float32),
        "x_att_nloc": np.ascontiguousarray(x_att[:, n0:n0 + NLOC, :]),
        "ident": np.eye(P, dtype=np.float32),
        "sel": sel,
        "c0": c0,
        "wg": wg, "whid": whid_l, "wq": wq_l,
        "wfixih": np.ascontiguousarray(w_ih[:, 0:64].T),
        "wfixhid": np.ascontiguousarray(whid[:, 256:320].T),
        "p1b": p1b, "p2b": p2b,
        "wdfix": np.ascontiguousarray(dw_ih[:, 0:64].T),
        "wdav": _chunks(np.ascontiguousarray(dw_ih[:, 64:576].T)),
        "wdhh": _chunks(np.ascontiguousarray(dw_hh.T)),
        "wfh": _chunks(np.ascontiguousarray(dh_w[:, 0:256].T)),
        "wffix": np.ascontiguousarray(dh_w[:, 256:320].T),
        "wfav": _chunks(np.ascontiguousarray(dh_w[:, 320:832].T)),
        "wa": np.ascontiguousarray(dh_w[:, 832:896].T),
        "w2t": _chunks(np.ascontiguousarray(
            np.asarray(gw["dec_out_w"], np.float32).T)),
        "dhb": np.ascontiguousarray(
            np.asarray(gw["dec_hid_b"], np.float32).reshape(2, P).T),
        "b2b": np.tile(np.asarray(gw["dec_out_b"], np.float32)[None, :],
                       (P, 1)),
    }


_CACHE = {}


def kernel(**inputs):
    T = int(np.asarray(inputs["x_fix"]).shape[0])
    HZ = int(np.asarray(inputs["train_horizon"]))
    N = int(np.asarray(inputs["x_att"]).shape[1])
    NLOC = N // N_CORES
    key = (T, HZ, NLOC)
    if key not in _CACHE:
        nc = build_program(T, HZ, NLOC, add_c0=True)
        nc.compile()
        _CACHE[key] = nc
    nc = _CACHE[key]
    in_maps = [_prep(inputs, NLOC, c * NLOC) for c in range(N_CORES)]
    res = bass_utils.run_bass_kernel_spmd(nc, in_maps,
                                          core_ids=list(range(N_CORES)))
    pred = np.concatenate([res.results[c]["pred"] for c in range(N_CORES)],
                          axis=1)
    r0 = res.results[0]
    global _LAST_R0
    _LAST_R0 = r0
    return (pred, r0["attw"], r0["q_gru_n"], r0["dec_gru_n"], r0["query_n"])


def timed_run(inputs):
    """Profiled run; returns HW exec time in ns."""
    T = int(np.asarray(inputs["x_fix"]).shape[0])
    HZ = int(np.asarray(inputs["train_horizon"]))
    N = int(np.asarray(inputs["x_att"]).shape[1])
    NLOC = N // N_CORES
    nc = _CACHE[(T, HZ, NLOC)]
    in_maps = [_prep(inputs, NLOC, c * NLOC) for c in range(N_CORES)]
    res = bass_utils.run_bass_kernel_spmd(
        nc, in_maps, core_ids=list(range(N_CORES)), trace=True)
    global _LAST_TRACE
    _LAST_TRACE = res
    return res.exec_time_ns
